# revision 5
# baseline (speedup 1.0000x reference)
"""Causal attention (B=8, S=2048, D=1024, fp32) on 8 TRN2 NeuronCores.

Sharding: batch-parallel, one batch element per core (SPMD, no collectives).

Per-core algorithm (S^T layout), v4:
  - Host casts Q/K/V to bf16 (identical rounding to an in-DMA cast), halving
    input HBM traffic; output is stored bf16 and upcast on host.
  - Q/K transposes are hybrid: groups 0-2 are staged + transposed on TensorE
    (their data is needed within the first ~20us), groups 3-7 go through the
    DMA engine's XBAR transpose straight from DRAM (measured ~52 GB/s -- too
    slow for everything, but its 5MB finish long before those groups start,
    and each PE transpose costs a full ~118ns LDWEIGHTS slot).
  - Scores are computed transposed: S^T[k, q] = sum_d KT[d,k] * QT[d,q],
    accumulated over 8 d-subtiles in PSUM, 2 k-tiles x 256 q per PSUM bank;
    k-tiles above the diagonal are skipped entirely.
  - exp(dots/sqrt(D)) on ScalarE, output bf16 = P^T; on the diagonal pair a
    multiplicative bf16 mask zeroes invalid entries and the fully-masked
    quarter is memset, not computed.
  - Row sums: P^T tiles are accumulated across pairs on DVE into fp32; at
    group end the two k-halves fold (bf16) and two tiny matmuls against a
    ones vector give per-q sums.  (Replaces one N=1 ones-matmul per
    pair/kk/j whose successor LDWEIGHTS was ~127 exposed PE cycles each.)
  - PV: O[q, d] += P^T.T @ V with V in native [k, d] layout; normalization
    multiplies by the reciprocal row sum; output stored bf16 on sync/scalar
    queues (final group split across both for a short tail).
  - Dummy matmuls at t~0 keep the PE busy during the DMA-bound startup so
    the HAM clock gate ramps to 8/8 before the real work lands.
"""

import numpy as np

import concourse.bass as bass
import concourse.mybir as mybir
import concourse.tile as tile
from concourse import bacc
from concourse.masks import make_identity

P = 128
PE_TRANS_GROUPS = 3    # groups transposed on TensorE; rest via XBAR DMA


def build_attention_nc(S=2048, D=1024):
    f32, bf16 = mybir.dt.float32, mybir.dt.bfloat16
    nc = bacc.Bacc(None, target_bir_lowering=False)

    q_d = nc.dram_tensor("query", [S, D], bf16, kind="ExternalInput")
    k_d = nc.dram_tensor("key", [S, D], bf16, kind="ExternalInput")
    v_d = nc.dram_tensor("value", [S, D], bf16, kind="ExternalInput")
    o_d = nc.dram_tensor("out", [S, D], bf16, kind="ExternalOutput")

    NT = S // P            # number of 128-row seq tiles
    ND = D // P            # number of 128-wide d subtiles
    QGT = 2                # q-tiles per group
    QG = QGT * P           # q-group width (256)
    NG = S // QG           # number of q groups
    DH = min(D, 512)       # PV free-dim chunk (one PSUM bank)
    NDH = D // DH
    TCH = 4 if ND % 4 == 0 else (2 if ND % 2 == 0 else 1)  # transpose chunk
    scale = 1.0 / float(np.sqrt(D))

    qv = q_d.rearrange("(n p) d -> p n d", p=P)
    kv = k_d.rearrange("(n p) d -> p n d", p=P)
    vv = v_d.rearrange("(n p) d -> p n d", p=P)
    ov = o_d.rearrange("(n p) d -> p n d", p=P)

    with tile.TileContext(nc) as tc:
        with (
            tc.tile_pool(name="const", bufs=1) as constp,
            tc.tile_pool(name="slab", bufs=1) as slab,
            tc.tile_pool(name="stage", bufs=8) as stagep,
            tc.tile_pool(name="pt", bufs=3) as ptp,
            tc.tile_pool(name="psum_sb", bufs=2) as psumsb,
            tc.tile_pool(name="small", bufs=2) as smallp,
            tc.tile_pool(name="ost", bufs=4) as ostp,
            tc.tile_pool(name="ps", bufs=1, space="PSUM") as psp,
        ):
            # Dep-free warm operands + ones vector (DVE so the DMA-issuing
            # queues are free from the first instruction).
            warmslab = constp.tile([P, 512], bf16)
            nc.vector.memset(warmslab[:], 0.0)
            ones = constp.tile([P, 1], bf16)
            nc.vector.memset(ones[:], 1.0)

            # Warm the PE clock gate (HAM) during the DMA-bound startup.
            for _ in range(10):
                warm = psp.tile([P, 512], f32, tag="st", bufs=3)
                nc.tensor.matmul(
                    warm[:], lhsT=warmslab[:, :P], rhs=warmslab[:],
                    start=True, stop=True,
                )

            ident = constp.tile([P, P], bf16)
            make_identity(nc, ident[:])

            # [d%128, seq_tile, d//128, s_in_tile]; each [:, t] slice is a
            # contiguous [P, ND, P] region (ds-major) -- the layout the XBAR
            # transpose writes and the PE-transpose copies match.
            QT = slab.tile([P, NT, ND, P], bf16)
            KT = slab.tile([P, NT, ND, P], bf16)
            V = slab.tile([P, NT, D], bf16)    # [k%128, k//128, d]

            def emit_stage_loads(g, fine=False):
                """Stage loads for a PE-transposed group: q on sync, k on
                gpsimd.  fine=True splits into half-D chunks so the first
                transposes start as soon as half a tile landed."""
                stages = {}
                for nm, srcv, eng in (("q", qv, nc.sync), ("k", kv, nc.gpsimd)):
                    for t in range(QGT * g, QGT * (g + 1)):
                        stg = stagep.tile([P, D], bf16, tag="stage", name=f"stg_{nm}{t}")
                        if fine:
                            hd = D // 2
                            eng.dma_start(stg[:, :hd], srcv[:, t, :hd])
                            eng.dma_start(stg[:, hd:], srcv[:, t, hd:])
                        else:
                            eng.dma_start(stg[:], srcv[:, t, :])
                        stages[(nm, t)] = stg
                return stages

            def emit_v_loads(g):
                for t in range(QGT * g, QGT * (g + 1)):
                    nc.gpsimd.dma_start(V[:, t, :], vv[:, t, :])

            def emit_pe_transposes(g, stages):
                # Q tiles first: group g's QK matmuls need QT immediately,
                # but the new KT tiles only at the diagonal (last) pair.
                for nm, dst in (("q", QT), ("k", KT)):
                    for t in range(QGT * g, QGT * (g + 1)):
                        stg = stages[(nm, t)]
                        for c in range(ND // TCH):
                            pst = psp.tile([P, TCH, P], bf16, tag="st", bufs=3)
                            for j in range(TCH):
                                ds = c * TCH + j
                                nc.tensor.transpose(
                                    pst[:, j, :],
                                    stg[:, ds * P : (ds + 1) * P],
                                    ident[:],
                                )
                            dslc = dst[:, t, c * TCH : (c + 1) * TCH, :]
                            if nm == "q":
                                nc.vector.tensor_copy(dslc, pst[:])
                            else:
                                nc.scalar.copy(dslc, pst[:])

            # Startup: stage loads for the PE-transposed groups...
            stage_of = {}
            stage_of[0] = emit_stage_loads(0, fine=True)
            emit_v_loads(0)
            for g in range(1, PE_TRANS_GROUPS):
                stage_of[g] = emit_stage_loads(g)
                emit_v_loads(g)

            # ...then the XBAR transposes for all later groups, in deadline
            # order (sync queue; issued behind the group-0..2 q loads).
            for g in range(PE_TRANS_GROUPS, NG):
                for t in range(QGT * g, QGT * (g + 1)):
                    nc.sync.dma_start_transpose(
                        QT[:, t], q_d[t * P : (t + 1) * P, :]
                    )
                for t in range(QGT * g, QGT * (g + 1)):
                    nc.sync.dma_start_transpose(
                        KT[:, t], k_d[t * P : (t + 1) * P, :]
                    )

            # Multiplicative bf16 causal mask for the diagonal k-tile pair,
            # S^T layout (1=valid, 0=masked), applied to P^T after exp.
            mask01 = constp.tile([P, 2, QG], bf16)
            for half in range(2):
                m = mask01[:, half, :]
                nc.gpsimd.memset(m, 1.0)
                nc.gpsimd.affine_select(
                    out=m,
                    in_=m,
                    compare_op=mybir.AluOpType.is_ge,
                    fill=0.0,
                    base=-(P * half),
                    pattern=[[1, QG]],
                    channel_multiplier=-1,
                )

            for g in range(NG):
                if g in stage_of:
                    emit_pe_transposes(g, stage_of.pop(g))
                if g + 2 < NG and g + 2 >= PE_TRANS_GROUPS:
                    emit_v_loads(g + 2)

                # ---- score + softmax + PV over k-tile pairs ----
                opv = [
                    [
                        psp.tile(
                            [P, DH], f32, tag=f"pv{j}_{dh}", bufs=1,
                            name=f"opv{j}_{dh}",
                        )
                        for dh in range(NDH)
                    ]
                    for j in range(QGT)
                ]
                # Running fp32 sum of P^T across this group's pairs (DVE).
                psum_p = psumsb.tile([P, 2, QG], f32, tag="psum_p")
                for p in range(g + 1):
                    diag = p == g
                    stps = psp.tile([P, 2, QG], f32, tag="st", bufs=3)
                    for kk in range(2):
                        ki = 2 * p + kk
                        # Diagonal pair, second k-tile: q < 128 (rel) is fully
                        # masked, so only compute the upper q half (N=128).
                        qlo = P if (diag and kk == 1) else 0
                        for ds in range(ND):
                            if qlo:
                                rhs = QT[:, 2 * g + 1, ds, :]
                            else:
                                rhs = QT[:, 2 * g : 2 * g + 2, ds, :]
                            nc.tensor.matmul(
                                stps[:, kk, qlo:],
                                lhsT=KT[:, ki, ds, :],
                                rhs=rhs,
                                start=(ds == 0),
                                stop=(ds == ND - 1),
                            )
                    if diag:
                        # The uncomputed quarter never got written: give it a
                        # finite value; the multiplicative mask below zeroes
                        # it (and all other masked entries) after exp.
                        nc.vector.memset(stps[:, 1, :P], 0.0)
                    ptt = ptp.tile([P, 2, QG], bf16, tag="pt")
                    nc.scalar.activation(
                        ptt[:], stps[:], mybir.ActivationFunctionType.Exp,
                        scale=scale,
                    )
                    if diag:
                        nc.vector.tensor_mul(ptt[:], ptt[:], mask01[:])
                    if p == 0:
                        nc.vector.tensor_copy(psum_p[:], ptt[:])
                    else:
                        nc.vector.tensor_add(psum_p[:], psum_p[:], ptt[:])
                    for kk in range(2):
                        ki = 2 * p + kk
                        first = (p == 0) and (kk == 0)
                        for j in range(QGT):
                            if diag and kk == 1 and j == 0:
                                continue  # fully masked block
                            # last matmul touching opv[j]'s accumulation:
                            last_j = diag and (kk == 1 or (kk == 0 and j == 0))
                            lh = ptt[:, kk, j * P : (j + 1) * P]
                            for dh in range(NDH):
                                nc.tensor.matmul(
                                    opv[j][dh][:],
                                    lhsT=lh,
                                    rhs=V[:, ki, dh * DH : (dh + 1) * DH],
                                    start=first,
                                    stop=last_j,
                                )

                # ---- row sums -> reciprocal -> normalize + store ----
                folded = psumsb.tile([P, QG], bf16, tag="folded")
                nc.vector.tensor_add(
                    folded[:], psum_p[:, 0, :], psum_p[:, 1, :]
                )
                rsps = psp.tile([P, QGT], f32, tag="rs", bufs=1)
                for j in range(QGT):
                    nc.tensor.matmul(
                        rsps[:, j : j + 1],
                        lhsT=folded[:, j * P : (j + 1) * P],
                        rhs=ones[:],
                        start=(j == 0),
                        stop=(j == QGT - 1),
                    )
                rec = smallp.tile([P, QGT], f32, tag="rec")
                nc.vector.reciprocal(rec[:], rsps[:])
                final = g == NG - 1
                for j in range(QGT):
                    ost = ostp.tile([P, D], bf16, tag="ost")
                    for dh in range(NDH):
                        osl = ost[:, dh * DH : (dh + 1) * DH]
                        if final and dh % 2 == 1:
                            nc.scalar.mul(osl, opv[j][dh][:], mul=rec[:, j : j + 1])
                        else:
                            nc.vector.tensor_scalar_mul(
                                osl, opv[j][dh][:], scalar1=rec[:, j : j + 1]
                            )
                        # Final group: split the last stores across two DMA
                        # queues so they issue in parallel (short tail).
                        eng = nc.scalar if (final and dh % 2 == 1) else nc.sync
                        eng.dma_start(
                            ov[:, g * QGT + j, dh * DH : (dh + 1) * DH], osl
                        )

    nc.compile()
    return nc


_NC_CACHE = {}


def _get_nc(S, D):
    if (S, D) not in _NC_CACHE:
        _NC_CACHE[(S, D)] = build_attention_nc(S, D)
    return _NC_CACHE[(S, D)]


def kernel(query, key, value):
    import ml_dtypes
    from concourse.bass_utils import run_bass_kernel_spmd

    bf = ml_dtypes.bfloat16
    query = np.asarray(query).astype(bf)
    key = np.asarray(key).astype(bf)
    value = np.asarray(value).astype(bf)
    B, S, D = query.shape
    nc = _get_nc(S, D)
    in_maps = [
        {
            "query": np.ascontiguousarray(query[i]),
            "key": np.ascontiguousarray(key[i]),
            "value": np.ascontiguousarray(value[i]),
        }
        for i in range(B)
    ]
    res = run_bass_kernel_spmd(nc, in_maps, core_ids=list(range(B)))
    out = np.stack([r["out"] for r in res.results], axis=0)
    return out.astype(np.float32)


# revision 6
# speedup vs baseline: 1.2982x; 1.2982x over previous
"""Causal attention (B=8, S=2048, D=1024, fp32) on 8 TRN2 NeuronCores.

Sharding: batch-parallel, one batch element per core (SPMD, no collectives).

Per-core algorithm (S^T layout):
  - Q, K are loaded with an fp32->bf16 cast during DMA, then transposed on
    TensorE (128x128 tiles vs a bf16 identity) into [d, s] layouts QT/KT.
  - Scores are computed transposed: S^T[k, q] = sum_d KT[d,k] * QT[d,q],
    accumulated over 8 d-subtiles in PSUM, 2 k-tiles x 256 q per PSUM bank.
  - Causal mask: additive -1e10 on the diagonal pair only (precomputed
    [128, 2, 256] mask via affine_select); k-tiles above the diagonal are
    skipped entirely.
  - exp((dots+mask)/sqrt(D)) on ScalarE (no max subtraction: |dots| <= ~1.1e3
    so logits <= ~35, exp fits fp32 comfortably), output cast to bf16 = P^T.
  - PV: O[q, d] += P^T.T @ V with V in native [k, d] layout; row sums via an
    extra N=1 matmul against a ones vector; final normalization is a DVE
    multiply by the reciprocal row sum (numerator/denominator both built from
    the same bf16 P^T, so rounding cancels to first order).
"""

import numpy as np

import concourse.bass as bass
import concourse.mybir as mybir
import concourse.tile as tile
from concourse import bacc
from concourse.masks import make_identity

P = 128
MASKVAL = -1e10  # matches reference INF (subtracted pre-scale)


def build_attention_nc(S=2048, D=1024):
    f32, bf16 = mybir.dt.float32, mybir.dt.bfloat16
    nc = bacc.Bacc(None, target_bir_lowering=False)

    q_d = nc.dram_tensor("query", [S, D], f32, kind="ExternalInput")
    k_d = nc.dram_tensor("key", [S, D], f32, kind="ExternalInput")
    v_d = nc.dram_tensor("value", [S, D], f32, kind="ExternalInput")
    o_d = nc.dram_tensor("out", [S, D], f32, kind="ExternalOutput")

    NT = S // P            # number of 128-row seq tiles
    ND = D // P            # number of 128-wide d subtiles
    QGT = 2                # q-tiles per group
    QG = QGT * P           # q-group width (256)
    NG = S // QG           # number of q groups
    DH = min(D, 512)       # PV free-dim chunk (one PSUM bank)
    NDH = D // DH
    TCH = 4 if ND % 4 == 0 else (2 if ND % 2 == 0 else 1)  # transpose chunk
    scale = 1.0 / float(np.sqrt(D))

    qv = q_d.rearrange("(n p) d -> p n d", p=P)
    kv = k_d.rearrange("(n p) d -> p n d", p=P)
    vv = v_d.rearrange("(n p) d -> p n d", p=P)
    ov = o_d.rearrange("(n p) d -> p n d", p=P)

    with tile.TileContext(nc) as tc:
        with (
            tc.tile_pool(name="const", bufs=1) as constp,
            tc.tile_pool(name="slab", bufs=1) as slab,
            tc.tile_pool(name="stage", bufs=8) as stagep,
            tc.tile_pool(name="pt", bufs=3) as ptp,
            tc.tile_pool(name="small", bufs=2) as smallp,
            tc.tile_pool(name="ost", bufs=2) as ostp,
            tc.tile_pool(name="ps", bufs=1, space="PSUM") as psp,
        ):
            ident = constp.tile([P, P], bf16)
            make_identity(nc, ident[:])
            ones = constp.tile([P, 1], bf16)
            nc.vector.memset(ones[:], 1.0)

            # Additive causal mask for the diagonal k-tile pair, S^T layout:
            # maskt[kk, half, qq] = 0 if (128*half + kk) <= qq else MASKVAL
            maskt = constp.tile([P, 2, QG], f32)
            for half in range(2):
                m = maskt[:, half, :]
                nc.gpsimd.memset(m, 0.0)
                nc.gpsimd.affine_select(
                    out=m,
                    in_=m,
                    compare_op=mybir.AluOpType.is_ge,
                    fill=MASKVAL,
                    base=-(P * half),
                    pattern=[[1, QG]],
                    channel_multiplier=-1,
                )
            # Multiplicative bf16 variant (1=valid, 0=masked), applied to P^T
            # after exp — cheaper than fp32 PSUM adds before exp.
            mask01 = constp.tile([P, 2, QG], bf16)
            for half in range(2):
                m = mask01[:, half, :]
                nc.gpsimd.memset(m, 1.0)
                nc.gpsimd.affine_select(
                    out=m,
                    in_=m,
                    compare_op=mybir.AluOpType.is_ge,
                    fill=0.0,
                    base=-(P * half),
                    pattern=[[1, QG]],
                    channel_multiplier=-1,
                )

            QT = slab.tile([P, ND, S], bf16)   # [d%128, d//128, q]
            KT = slab.tile([P, ND, S], bf16)   # [d%128, d//128, k]
            V = slab.tile([P, NT, D], bf16)    # [k%128, k//128, d]

            # Warm the PE clock gate (HAM) during the DMA-bound startup:
            # fp32 matmuls on the mask tile have no DMA dependency and give
            # ~4096 busy PE cycles, flipping the gate to 8/8 before the
            # first real matmuls arrive.
            warm = psp.tile([P, QG], f32, tag="st", bufs=3)
            for _ in range(4):
                nc.tensor.matmul(
                    warm[:], lhsT=maskt[:, 0, :P], rhs=maskt[:, 0, :],
                    start=True, stop=True,
                )

            def emit_loads(g):
                """Issue the cast-DMAs for group g's new Q/K/V tiles.

                Loads are issued in the order transposes consume them (all Q
                tiles, then K tiles), in half-D chunks so the first 4 subtile
                transposes can start as soon as half a tile has landed.
                """
                stages = {}
                for nm, srcv in (("q", qv), ("k", kv)):
                    for t in range(QGT * g, QGT * (g + 1)):
                        stg = stagep.tile([P, D], bf16, tag="stage", name=f"stg_{nm}{t}")
                        hd = D // 2
                        nc.gpsimd.dma_start(stg[:, :hd], srcv[:, t, :hd])
                        nc.gpsimd.dma_start(stg[:, hd:], srcv[:, t, hd:])
                        stages[(nm, t)] = stg
                for t in range(QGT * g, QGT * (g + 1)):
                    nc.gpsimd.dma_start(V[:, t, :], vv[:, t, :])  # fp32->bf16
                return stages

            def emit_transposes(g, stages):
                # Q tiles first: group g's QK matmuls need QT immediately,
                # but the new KT tiles only at the diagonal (last) pair.
                for nm, dst in (("q", QT), ("k", KT)):
                    for t in range(QGT * g, QGT * (g + 1)):
                        stg = stages[(nm, t)]
                        for c in range(ND // TCH):
                            pst = psp.tile([P, TCH, P], bf16, tag="st", bufs=3)
                            for j in range(TCH):
                                ds = c * TCH + j
                                nc.tensor.transpose(
                                    pst[:, j, :],
                                    stg[:, ds * P : (ds + 1) * P],
                                    ident[:],
                                )
                            dslc = dst[:, c * TCH : (c + 1) * TCH, t * P : (t + 1) * P]
                            if nm == "q":
                                nc.vector.tensor_copy(dslc, pst[:])
                            else:
                                nc.scalar.copy(dslc, pst[:])

            pending = emit_loads(0)
            for g in range(NG):
                # Prefetch next group's DMA loads before anything else so
                # they land while this group's pair loop runs.
                nxt = emit_loads(g + 1) if g + 1 < NG else None
                emit_transposes(g, pending)
                pending = nxt

                # ---- score + softmax + PV over k-tile pairs ----
                # One PSUM tile per (q-tile, d-half) so each bank is released
                # as soon as its own normalize-read completes.
                opv = [
                    [
                        psp.tile(
                            [P, DH], f32, tag=f"pv{j}_{dh}", bufs=1,
                            name=f"opv{j}_{dh}",
                        )
                        for dh in range(NDH)
                    ]
                    for j in range(QGT)
                ]
                rsps = psp.tile([P, QGT], f32, tag="rs", bufs=1)
                for p in range(g + 1):
                    diag = p == g
                    stps = psp.tile([P, 2, QG], f32, tag="st", bufs=3)
                    for kk in range(2):
                        ki = 2 * p + kk
                        # Diagonal pair, second k-tile: q < 128 (rel) is fully
                        # masked, so only compute the upper q half (N=128).
                        qlo = P if (diag and kk == 1) else 0
                        for ds in range(ND):
                            nc.tensor.matmul(
                                stps[:, kk, qlo:],
                                lhsT=KT[:, ds, ki * P : (ki + 1) * P],
                                rhs=QT[:, ds, g * QG + qlo : (g + 1) * QG],
                                start=(ds == 0),
                                stop=(ds == ND - 1),
                            )
                    if diag:
                        # The uncomputed quarter never got written: give it a
                        # finite value; the multiplicative mask below zeroes
                        # it (and all other masked entries) after exp.
                        nc.vector.memset(stps[:, 1, :P], 0.0)
                    ptt = ptp.tile([P, 2, QG], bf16, tag="pt")
                    nc.scalar.activation(
                        ptt[:], stps[:], mybir.ActivationFunctionType.Exp,
                        scale=scale,
                    )
                    if diag:
                        nc.vector.tensor_mul(ptt[:], ptt[:], mask01[:])
                    for kk in range(2):
                        ki = 2 * p + kk
                        first = (p == 0) and (kk == 0)
                        for j in range(QGT):
                            if diag and kk == 1 and j == 0:
                                continue  # fully masked block
                            # last matmul touching opv[j]'s accumulation:
                            last_j = diag and (kk == 1 or (kk == 0 and j == 0))
                            lh = ptt[:, kk, j * P : (j + 1) * P]
                            for dh in range(NDH):
                                nc.tensor.matmul(
                                    opv[j][dh][:],
                                    lhsT=lh,
                                    rhs=V[:, ki, dh * DH : (dh + 1) * DH],
                                    start=first,
                                    stop=last_j,
                                )
                            # rsps is one PSUM bank = one zero region: start
                            # exactly once (marks whole bank pending-zero, so
                            # each column's first write lands as overwrite).
                            nc.tensor.matmul(
                                rsps[:, j : j + 1],
                                lhsT=lh,
                                rhs=ones[:],
                                start=(first and j == 0),
                                stop=(diag and kk == 1 and j == QGT - 1),
                            )

                # ---- normalize + store (per d-half, shipping each half as
                # soon as it is scaled; final group splits across DVE+ACT
                # since no later exp can be delayed) ----
                rec = smallp.tile([P, QGT], f32, tag="rec")
                nc.vector.reciprocal(rec[:], rsps[:])
                final = g == NG - 1
                for j in range(QGT):
                    ost = ostp.tile([P, D], f32, tag="ost")
                    for dh in range(NDH):
                        osl = ost[:, dh * DH : (dh + 1) * DH]
                        if final and dh % 2 == 1:
                            nc.scalar.mul(osl, opv[j][dh][:], mul=rec[:, j : j + 1])
                        else:
                            nc.vector.tensor_scalar_mul(
                                osl, opv[j][dh][:], scalar1=rec[:, j : j + 1]
                            )
                        nc.sync.dma_start(
                            ov[:, g * QGT + j, dh * DH : (dh + 1) * DH], osl
                        )

    nc.compile()
    return nc


_NC_CACHE = {}


def _get_nc(S, D):
    if (S, D) not in _NC_CACHE:
        _NC_CACHE[(S, D)] = build_attention_nc(S, D)
    return _NC_CACHE[(S, D)]


def kernel(query, key, value):
    from concourse.bass_utils import run_bass_kernel_spmd

    query = np.asarray(query, dtype=np.float32)
    key = np.asarray(key, dtype=np.float32)
    value = np.asarray(value, dtype=np.float32)
    B, S, D = query.shape
    nc = _get_nc(S, D)
    in_maps = [
        {
            "query": np.ascontiguousarray(query[i]),
            "key": np.ascontiguousarray(key[i]),
            "value": np.ascontiguousarray(value[i]),
        }
        for i in range(B)
    ]
    res = run_bass_kernel_spmd(nc, in_maps, core_ids=list(range(B)))
    out = np.stack([r["out"] for r in res.results], axis=0)
    return out.astype(np.float32)



# revision 7
# speedup vs baseline: 1.5732x; 1.2118x over previous
"""Causal attention (B=8, S=2048, D=1024, fp32) on 8 TRN2 NeuronCores.

Sharding: batch-parallel, one batch element per core (SPMD, no collectives).

Per-core algorithm (S^T layout):
  - Host casts Q/K/V to bf16 (same rounding as an in-DMA cast would apply);
    output is stored bf16 and upcast on host.  Q, K are then transposed on
    TensorE (128x128 tiles vs a bf16 identity) into [d, s] layouts QT/KT.
  - Scores are computed transposed: S^T[k, q] = sum_d KT[d,k] * QT[d,q],
    accumulated over 8 d-subtiles in PSUM, 2 k-tiles x 256 q per PSUM bank.
  - Causal mask: additive -1e10 on the diagonal pair only (precomputed
    [128, 2, 256] mask via affine_select); k-tiles above the diagonal are
    skipped entirely.
  - exp((dots+mask)/sqrt(D)) on ScalarE (no max subtraction: |dots| <= ~1.1e3
    so logits <= ~35, exp fits fp32 comfortably), output cast to bf16 = P^T.
  - PV: O[q, d] += P^T.T @ V with V in native [k, d] layout; row sums via an
    extra N=1 matmul against a ones vector; final normalization is a DVE
    multiply by the reciprocal row sum (numerator/denominator both built from
    the same bf16 P^T, so rounding cancels to first order).
"""

import numpy as np

import concourse.bass as bass
import concourse.mybir as mybir
import concourse.tile as tile
from concourse import bacc
from concourse.masks import make_identity

P = 128
MASKVAL = -1e10  # matches reference INF (subtracted pre-scale)


def build_attention_nc(S=2048, D=1024):
    f32, bf16 = mybir.dt.float32, mybir.dt.bfloat16
    nc = bacc.Bacc(None, target_bir_lowering=False)

    q_d = nc.dram_tensor("query", [S, D], bf16, kind="ExternalInput")
    k_d = nc.dram_tensor("key", [S, D], bf16, kind="ExternalInput")
    v_d = nc.dram_tensor("value", [S, D], bf16, kind="ExternalInput")
    o_d = nc.dram_tensor("out", [S, D], bf16, kind="ExternalOutput")

    NT = S // P            # number of 128-row seq tiles
    ND = D // P            # number of 128-wide d subtiles
    QGT = 2                # q-tiles per group
    QG = QGT * P           # q-group width (256)
    NG = S // QG           # number of q groups
    DH = min(D, 512)       # PV free-dim chunk (one PSUM bank)
    NDH = D // DH
    TCH = 4 if ND % 4 == 0 else (2 if ND % 2 == 0 else 1)  # transpose chunk
    scale = 1.0 / float(np.sqrt(D))

    qv = q_d.rearrange("(n p) d -> p n d", p=P)
    kv = k_d.rearrange("(n p) d -> p n d", p=P)
    vv = v_d.rearrange("(n p) d -> p n d", p=P)
    ov = o_d.rearrange("(n p) d -> p n d", p=P)

    with tile.TileContext(nc) as tc:
        with (
            tc.tile_pool(name="const", bufs=1) as constp,
            tc.tile_pool(name="slab", bufs=1) as slab,
            tc.tile_pool(name="stage", bufs=8) as stagep,
            tc.tile_pool(name="pt", bufs=3) as ptp,
            tc.tile_pool(name="small", bufs=2) as smallp,
            tc.tile_pool(name="ost", bufs=2) as ostp,
            tc.tile_pool(name="ps", bufs=1, space="PSUM") as psp,
        ):
            ident = constp.tile([P, P], bf16)
            make_identity(nc, ident[:])
            ones = constp.tile([P, 1], bf16)
            nc.vector.memset(ones[:], 1.0)

            # Additive causal mask for the diagonal k-tile pair, S^T layout:
            # maskt[kk, half, qq] = 0 if (128*half + kk) <= qq else MASKVAL
            maskt = constp.tile([P, 2, QG], f32)
            for half in range(2):
                m = maskt[:, half, :]
                nc.gpsimd.memset(m, 0.0)
                nc.gpsimd.affine_select(
                    out=m,
                    in_=m,
                    compare_op=mybir.AluOpType.is_ge,
                    fill=MASKVAL,
                    base=-(P * half),
                    pattern=[[1, QG]],
                    channel_multiplier=-1,
                )
            # Multiplicative bf16 variant (1=valid, 0=masked), applied to P^T
            # after exp — cheaper than fp32 PSUM adds before exp.
            mask01 = constp.tile([P, 2, QG], bf16)
            for half in range(2):
                m = mask01[:, half, :]
                nc.gpsimd.memset(m, 1.0)
                nc.gpsimd.affine_select(
                    out=m,
                    in_=m,
                    compare_op=mybir.AluOpType.is_ge,
                    fill=0.0,
                    base=-(P * half),
                    pattern=[[1, QG]],
                    channel_multiplier=-1,
                )

            QT = slab.tile([P, ND, S], bf16)   # [d%128, d//128, q]
            KT = slab.tile([P, ND, S], bf16)   # [d%128, d//128, k]
            V = slab.tile([P, NT, D], bf16)    # [k%128, k//128, d]

            # Warm the PE clock gate (HAM) during the DMA-bound startup:
            # fp32 matmuls on the mask tile have no DMA dependency and give
            # ~4096 busy PE cycles, flipping the gate to 8/8 before the
            # first real matmuls arrive.
            warm = psp.tile([P, QG], f32, tag="st", bufs=3)
            for _ in range(4):
                nc.tensor.matmul(
                    warm[:], lhsT=maskt[:, 0, :P], rhs=maskt[:, 0, :],
                    start=True, stop=True,
                )

            def emit_loads(g):
                """Issue the cast-DMAs for group g's new Q/K/V tiles.

                Loads are issued in the order transposes consume them (all Q
                tiles, then K tiles), in half-D chunks so the first 4 subtile
                transposes can start as soon as half a tile has landed.
                """
                stages = {}
                for nm, srcv in (("q", qv), ("k", kv)):
                    for t in range(QGT * g, QGT * (g + 1)):
                        stg = stagep.tile([P, D], bf16, tag="stage", name=f"stg_{nm}{t}")
                        hd = D // 2
                        nc.gpsimd.dma_start(stg[:, :hd], srcv[:, t, :hd])
                        nc.gpsimd.dma_start(stg[:, hd:], srcv[:, t, hd:])
                        stages[(nm, t)] = stg
                for t in range(QGT * g, QGT * (g + 1)):
                    nc.gpsimd.dma_start(V[:, t, :], vv[:, t, :])
                return stages

            def emit_transposes(g, stages):
                # Q tiles first: group g's QK matmuls need QT immediately,
                # but the new KT tiles only at the diagonal (last) pair.
                for nm, dst in (("q", QT), ("k", KT)):
                    for t in range(QGT * g, QGT * (g + 1)):
                        stg = stages[(nm, t)]
                        for c in range(ND // TCH):
                            pst = psp.tile([P, TCH, P], bf16, tag="st", bufs=3)
                            for j in range(TCH):
                                ds = c * TCH + j
                                nc.tensor.transpose(
                                    pst[:, j, :],
                                    stg[:, ds * P : (ds + 1) * P],
                                    ident[:],
                                )
                            dslc = dst[:, c * TCH : (c + 1) * TCH, t * P : (t + 1) * P]
                            if nm == "q":
                                nc.vector.tensor_copy(dslc, pst[:])
                            else:
                                nc.scalar.copy(dslc, pst[:])

            pending = emit_loads(0)
            for g in range(NG):
                # Prefetch next group's DMA loads before anything else so
                # they land while this group's pair loop runs.
                nxt = emit_loads(g + 1) if g + 1 < NG else None
                emit_transposes(g, pending)
                pending = nxt

                # ---- score + softmax + PV over k-tile pairs ----
                # One PSUM tile per (q-tile, d-half) so each bank is released
                # as soon as its own normalize-read completes.
                opv = [
                    [
                        psp.tile(
                            [P, DH], f32, tag=f"pv{j}_{dh}", bufs=1,
                            name=f"opv{j}_{dh}",
                        )
                        for dh in range(NDH)
                    ]
                    for j in range(QGT)
                ]
                rsps = psp.tile([P, QGT], f32, tag="rs", bufs=1)
                for p in range(g + 1):
                    diag = p == g
                    stps = psp.tile([P, 2, QG], f32, tag="st", bufs=3)
                    for kk in range(2):
                        ki = 2 * p + kk
                        # Diagonal pair, second k-tile: q < 128 (rel) is fully
                        # masked, so only compute the upper q half (N=128).
                        qlo = P if (diag and kk == 1) else 0
                        for ds in range(ND):
                            nc.tensor.matmul(
                                stps[:, kk, qlo:],
                                lhsT=KT[:, ds, ki * P : (ki + 1) * P],
                                rhs=QT[:, ds, g * QG + qlo : (g + 1) * QG],
                                start=(ds == 0),
                                stop=(ds == ND - 1),
                            )
                    if diag:
                        # The uncomputed quarter never got written: give it a
                        # finite value; the multiplicative mask below zeroes
                        # it (and all other masked entries) after exp.
                        nc.vector.memset(stps[:, 1, :P], 0.0)
                    ptt = ptp.tile([P, 2, QG], bf16, tag="pt")
                    nc.scalar.activation(
                        ptt[:], stps[:], mybir.ActivationFunctionType.Exp,
                        scale=scale,
                    )
                    if diag:
                        nc.vector.tensor_mul(ptt[:], ptt[:], mask01[:])
                    for kk in range(2):
                        ki = 2 * p + kk
                        first = (p == 0) and (kk == 0)
                        for j in range(QGT):
                            if diag and kk == 1 and j == 0:
                                continue  # fully masked block
                            # last matmul touching opv[j]'s accumulation:
                            last_j = diag and (kk == 1 or (kk == 0 and j == 0))
                            lh = ptt[:, kk, j * P : (j + 1) * P]
                            for dh in range(NDH):
                                nc.tensor.matmul(
                                    opv[j][dh][:],
                                    lhsT=lh,
                                    rhs=V[:, ki, dh * DH : (dh + 1) * DH],
                                    start=first,
                                    stop=last_j,
                                )
                            # rsps is one PSUM bank = one zero region: start
                            # exactly once (marks whole bank pending-zero, so
                            # each column's first write lands as overwrite).
                            nc.tensor.matmul(
                                rsps[:, j : j + 1],
                                lhsT=lh,
                                rhs=ones[:],
                                start=(first and j == 0),
                                stop=(diag and kk == 1 and j == QGT - 1),
                            )

                # ---- normalize + store (per d-half, shipping each half as
                # soon as it is scaled; final group splits across DVE+ACT
                # since no later exp can be delayed) ----
                rec = smallp.tile([P, QGT], f32, tag="rec")
                nc.vector.reciprocal(rec[:], rsps[:])
                final = g == NG - 1
                for j in range(QGT):
                    ost = ostp.tile([P, D], bf16, tag="ost")
                    for dh in range(NDH):
                        osl = ost[:, dh * DH : (dh + 1) * DH]
                        if final and dh % 2 == 1:
                            nc.scalar.mul(osl, opv[j][dh][:], mul=rec[:, j : j + 1])
                        else:
                            nc.vector.tensor_scalar_mul(
                                osl, opv[j][dh][:], scalar1=rec[:, j : j + 1]
                            )
                        nc.sync.dma_start(
                            ov[:, g * QGT + j, dh * DH : (dh + 1) * DH], osl
                        )

    nc.compile()
    return nc


_NC_CACHE = {}


def _get_nc(S, D):
    if (S, D) not in _NC_CACHE:
        _NC_CACHE[(S, D)] = build_attention_nc(S, D)
    return _NC_CACHE[(S, D)]


def kernel(query, key, value):
    import ml_dtypes
    from concourse.bass_utils import run_bass_kernel_spmd

    bf = ml_dtypes.bfloat16
    query = np.asarray(query).astype(bf)
    key = np.asarray(key).astype(bf)
    value = np.asarray(value).astype(bf)
    B, S, D = query.shape
    nc = _get_nc(S, D)
    in_maps = [
        {
            "query": np.ascontiguousarray(query[i]),
            "key": np.ascontiguousarray(key[i]),
            "value": np.ascontiguousarray(value[i]),
        }
        for i in range(B)
    ]
    res = run_bass_kernel_spmd(nc, in_maps, core_ids=list(range(B)))
    out = np.stack([r["out"] for r in res.results], axis=0)
    return out.astype(np.float32)



# revision 8
# speedup vs baseline: 1.5977x; 1.0156x over previous
"""Causal attention (B=8, S=2048, D=1024, fp32) on 8 TRN2 NeuronCores.

Sharding: batch-parallel, one batch element per core (SPMD, no collectives).

Per-core algorithm (S^T layout):
  - Host casts Q/K/V to bf16 (same rounding as an in-DMA cast would apply);
    output is stored bf16 and upcast on host.  Q, K are then transposed on
    TensorE (128x128 tiles vs a bf16 identity) into [d, s] layouts QT/KT.
  - Scores are computed transposed: S^T[k, q] = sum_d KT[d,k] * QT[d,q],
    accumulated over 8 d-subtiles in PSUM, 2 k-tiles x 256 q per PSUM bank.
  - Causal mask: additive -1e10 on the diagonal pair only (precomputed
    [128, 2, 256] mask via affine_select); k-tiles above the diagonal are
    skipped entirely.
  - exp((dots+mask)/sqrt(D)) on ScalarE (no max subtraction: |dots| <= ~1.1e3
    so logits <= ~35, exp fits fp32 comfortably), output cast to bf16 = P^T.
  - PV: O[q, d] += P^T.T @ V with V in native [k, d] layout; row sums by
    accumulating P^T across pairs on DVE (fp32), folding the two k-halves,
    and two tiny ones-matmuls per group; final normalization is a DVE
    multiply by the reciprocal row sum (numerator/denominator both built from
    the same bf16 P^T, so rounding cancels to first order).
"""

import numpy as np

import concourse.bass as bass
import concourse.mybir as mybir
import concourse.tile as tile
from concourse import bacc
from concourse.masks import make_identity

P = 128
MASKVAL = -1e10  # matches reference INF (subtracted pre-scale)


def build_attention_nc(S=2048, D=1024):
    f32, bf16 = mybir.dt.float32, mybir.dt.bfloat16
    nc = bacc.Bacc(None, target_bir_lowering=False)

    q_d = nc.dram_tensor("query", [S, D], bf16, kind="ExternalInput")
    k_d = nc.dram_tensor("key", [S, D], bf16, kind="ExternalInput")
    v_d = nc.dram_tensor("value", [S, D], bf16, kind="ExternalInput")
    o_d = nc.dram_tensor("out", [S, D], bf16, kind="ExternalOutput")

    NT = S // P            # number of 128-row seq tiles
    ND = D // P            # number of 128-wide d subtiles
    QGT = 2                # q-tiles per group
    QG = QGT * P           # q-group width (256)
    NG = S // QG           # number of q groups
    DH = min(D, 512)       # PV free-dim chunk (one PSUM bank)
    NDH = D // DH
    TCH = 4 if ND % 4 == 0 else (2 if ND % 2 == 0 else 1)  # transpose chunk
    scale = 1.0 / float(np.sqrt(D))

    qv = q_d.rearrange("(n p) d -> p n d", p=P)
    kv = k_d.rearrange("(n p) d -> p n d", p=P)
    vv = v_d.rearrange("(n p) d -> p n d", p=P)
    ov = o_d.rearrange("(n p) d -> p n d", p=P)

    with tile.TileContext(nc) as tc:
        with (
            tc.tile_pool(name="const", bufs=1) as constp,
            tc.tile_pool(name="slab", bufs=1) as slab,
            tc.tile_pool(name="stage", bufs=8) as stagep,
            tc.tile_pool(name="pt", bufs=3) as ptp,
            tc.tile_pool(name="psum_sb", bufs=2) as psumsb,
            tc.tile_pool(name="small", bufs=2) as smallp,
            tc.tile_pool(name="ost", bufs=2) as ostp,
            tc.tile_pool(name="ps", bufs=1, space="PSUM") as psp,
        ):
            ident = constp.tile([P, P], bf16)
            make_identity(nc, ident[:])
            ones = constp.tile([P, 1], bf16)
            nc.vector.memset(ones[:], 1.0)

            # Additive causal mask for the diagonal k-tile pair, S^T layout:
            # maskt[kk, half, qq] = 0 if (128*half + kk) <= qq else MASKVAL
            maskt = constp.tile([P, 2, QG], f32)
            for half in range(2):
                m = maskt[:, half, :]
                nc.gpsimd.memset(m, 0.0)
                nc.gpsimd.affine_select(
                    out=m,
                    in_=m,
                    compare_op=mybir.AluOpType.is_ge,
                    fill=MASKVAL,
                    base=-(P * half),
                    pattern=[[1, QG]],
                    channel_multiplier=-1,
                )
            # Multiplicative bf16 variant (1=valid, 0=masked), applied to P^T
            # after exp — cheaper than fp32 PSUM adds before exp.
            mask01 = constp.tile([P, 2, QG], bf16)
            for half in range(2):
                m = mask01[:, half, :]
                nc.gpsimd.memset(m, 1.0)
                nc.gpsimd.affine_select(
                    out=m,
                    in_=m,
                    compare_op=mybir.AluOpType.is_ge,
                    fill=0.0,
                    base=-(P * half),
                    pattern=[[1, QG]],
                    channel_multiplier=-1,
                )

            QT = slab.tile([P, ND, S], bf16)   # [d%128, d//128, q]
            KT = slab.tile([P, ND, S], bf16)   # [d%128, d//128, k]
            V = slab.tile([P, NT, D], bf16)    # [k%128, k//128, d]

            # Warm the PE clock gate (HAM) during the DMA-bound startup:
            # fp32 matmuls on the mask tile have no DMA dependency and give
            # ~4096 busy PE cycles, flipping the gate to 8/8 before the
            # first real matmuls arrive.
            warm = psp.tile([P, QG], f32, tag="st", bufs=3)
            for _ in range(4):
                nc.tensor.matmul(
                    warm[:], lhsT=maskt[:, 0, :P], rhs=maskt[:, 0, :],
                    start=True, stop=True,
                )

            def emit_loads(g):
                """Issue the cast-DMAs for group g's new Q/K/V tiles.

                Loads are issued in the order transposes consume them (all Q
                tiles, then K tiles), in half-D chunks so the first 4 subtile
                transposes can start as soon as half a tile has landed.
                """
                stages = {}
                for nm, srcv in (("q", qv), ("k", kv)):
                    for t in range(QGT * g, QGT * (g + 1)):
                        stg = stagep.tile([P, D], bf16, tag="stage", name=f"stg_{nm}{t}")
                        hd = D // 2
                        nc.gpsimd.dma_start(stg[:, :hd], srcv[:, t, :hd])
                        nc.gpsimd.dma_start(stg[:, hd:], srcv[:, t, hd:])
                        stages[(nm, t)] = stg
                for t in range(QGT * g, QGT * (g + 1)):
                    nc.gpsimd.dma_start(V[:, t, :], vv[:, t, :])
                return stages

            def emit_transposes(g, stages):
                # Q tiles first: group g's QK matmuls need QT immediately,
                # but the new KT tiles only at the diagonal (last) pair.
                for nm, dst in (("q", QT), ("k", KT)):
                    for t in range(QGT * g, QGT * (g + 1)):
                        stg = stages[(nm, t)]
                        for c in range(ND // TCH):
                            pst = psp.tile([P, TCH, P], bf16, tag="st", bufs=3)
                            for j in range(TCH):
                                ds = c * TCH + j
                                nc.tensor.transpose(
                                    pst[:, j, :],
                                    stg[:, ds * P : (ds + 1) * P],
                                    ident[:],
                                )
                            dslc = dst[:, c * TCH : (c + 1) * TCH, t * P : (t + 1) * P]
                            if nm == "q":
                                nc.vector.tensor_copy(dslc, pst[:])
                            else:
                                nc.scalar.copy(dslc, pst[:])

            pending = emit_loads(0)
            for g in range(NG):
                # Prefetch next group's DMA loads before anything else so
                # they land while this group's pair loop runs.
                nxt = emit_loads(g + 1) if g + 1 < NG else None
                emit_transposes(g, pending)
                pending = nxt

                # ---- score + softmax + PV over k-tile pairs ----
                # One PSUM tile per (q-tile, d-half) so each bank is released
                # as soon as its own normalize-read completes.
                opv = [
                    [
                        psp.tile(
                            [P, DH], f32, tag=f"pv{j}_{dh}", bufs=1,
                            name=f"opv{j}_{dh}",
                        )
                        for dh in range(NDH)
                    ]
                    for j in range(QGT)
                ]
                # Running fp32 sum of P^T across this group's pairs (DVE).
                psum_p = psumsb.tile([P, 2, QG], f32, tag="psum_p")
                for p in range(g + 1):
                    diag = p == g
                    stps = psp.tile([P, 2, QG], f32, tag="st", bufs=3)
                    for kk in range(2):
                        ki = 2 * p + kk
                        # Diagonal pair, second k-tile: q < 128 (rel) is fully
                        # masked, so only compute the upper q half (N=128).
                        qlo = P if (diag and kk == 1) else 0
                        for ds in range(ND):
                            nc.tensor.matmul(
                                stps[:, kk, qlo:],
                                lhsT=KT[:, ds, ki * P : (ki + 1) * P],
                                rhs=QT[:, ds, g * QG + qlo : (g + 1) * QG],
                                start=(ds == 0),
                                stop=(ds == ND - 1),
                            )
                    if diag:
                        # The uncomputed quarter never got written: give it a
                        # finite value; the multiplicative mask below zeroes
                        # it (and all other masked entries) after exp.
                        nc.vector.memset(stps[:, 1, :P], 0.0)
                    ptt = ptp.tile([P, 2, QG], bf16, tag="pt")
                    nc.scalar.activation(
                        ptt[:], stps[:], mybir.ActivationFunctionType.Exp,
                        scale=scale,
                    )
                    if diag:
                        nc.vector.tensor_mul(ptt[:], ptt[:], mask01[:])
                    if p == 0:
                        nc.vector.tensor_copy(psum_p[:], ptt[:])
                    else:
                        nc.vector.tensor_add(psum_p[:], psum_p[:], ptt[:])
                    for kk in range(2):
                        ki = 2 * p + kk
                        first = (p == 0) and (kk == 0)
                        for j in range(QGT):
                            if diag and kk == 1 and j == 0:
                                continue  # fully masked block
                            # last matmul touching opv[j]'s accumulation:
                            last_j = diag and (kk == 1 or (kk == 0 and j == 0))
                            lh = ptt[:, kk, j * P : (j + 1) * P]
                            for dh in range(NDH):
                                nc.tensor.matmul(
                                    opv[j][dh][:],
                                    lhsT=lh,
                                    rhs=V[:, ki, dh * DH : (dh + 1) * DH],
                                    start=first,
                                    stop=last_j,
                                )

                # ---- row sums -> reciprocal -> normalize + store (per
                # d-half; final group splits across DVE+ACT since no later
                # exp can be delayed) ----
                folded = psumsb.tile([P, QG], bf16, tag="folded")
                nc.vector.tensor_add(
                    folded[:], psum_p[:, 0, :], psum_p[:, 1, :]
                )
                rsps = psp.tile([P, QGT], f32, tag="rs", bufs=1)
                for j in range(QGT):
                    nc.tensor.matmul(
                        rsps[:, j : j + 1],
                        lhsT=folded[:, j * P : (j + 1) * P],
                        rhs=ones[:],
                        start=(j == 0),
                        stop=(j == QGT - 1),
                    )
                rec = smallp.tile([P, QGT], f32, tag="rec")
                nc.vector.reciprocal(rec[:], rsps[:])
                final = g == NG - 1
                for j in range(QGT):
                    ost = ostp.tile([P, D], bf16, tag="ost")
                    for dh in range(NDH):
                        osl = ost[:, dh * DH : (dh + 1) * DH]
                        if final and dh % 2 == 1:
                            nc.scalar.mul(osl, opv[j][dh][:], mul=rec[:, j : j + 1])
                        else:
                            nc.vector.tensor_scalar_mul(
                                osl, opv[j][dh][:], scalar1=rec[:, j : j + 1]
                            )
                        nc.sync.dma_start(
                            ov[:, g * QGT + j, dh * DH : (dh + 1) * DH], osl
                        )

    nc.compile()
    return nc


_NC_CACHE = {}


def _get_nc(S, D):
    if (S, D) not in _NC_CACHE:
        _NC_CACHE[(S, D)] = build_attention_nc(S, D)
    return _NC_CACHE[(S, D)]


def kernel(query, key, value):
    import ml_dtypes
    from concourse.bass_utils import run_bass_kernel_spmd

    bf = ml_dtypes.bfloat16
    query = np.asarray(query).astype(bf)
    key = np.asarray(key).astype(bf)
    value = np.asarray(value).astype(bf)
    B, S, D = query.shape
    nc = _get_nc(S, D)
    in_maps = [
        {
            "query": np.ascontiguousarray(query[i]),
            "key": np.ascontiguousarray(key[i]),
            "value": np.ascontiguousarray(value[i]),
        }
        for i in range(B)
    ]
    res = run_bass_kernel_spmd(nc, in_maps, core_ids=list(range(B)))
    out = np.stack([r["out"] for r in res.results], axis=0)
    return out.astype(np.float32)

